# revision 7
# baseline (speedup 1.0000x reference)
"""Trainium2 Bass kernel for nn_Attention_6545530159375.

Full prefill attention (rope + GQA causal attention + output proj),
sharded over 8 NeuronCores as DP(batch=2) x TP(head-groups=4); TP
partials are summed on the host (no on-device collective needed).

Per core (batch b, shard j): 8 q heads, 2 kv heads, full sequence.
 - One pass over x computes the q, k AND v projections (x tiles stay
   resident in SBUF per 512-row q-tile and feed all three).
 - RoPE runs in SBUF in f16: the real/imag partition swap is a single
   DVE stream_shuffle (16-lane-interleaved head-dim layout baked into
   wq/wk/cos/sin on the host), no DRAM round trip.
 - Attention: scores matmul'd per 128-k-chunk into 2-bank PSUM groups,
   exp'd in ONE ScalarE activation per group (diagonal groups place
   their two valid score regions contiguously so they also take a
   single exp); causal masking via column-prefix trimming + one
   triangular f16 multiply per diagonal block; denominator via
   ones-matmul on the accumulated p tiles; reciprocal+normalize on
   DVE.
 - Output projection interleaved with attention per q-tile to keep PE
   dense while ScalarE/DVE drain softmax work; f16 output.
PSUM is one shared pool for both phases (tags A,B = 2 banks; c..f = 1
bank each) so the phase boundary pipelines instead of barriers; the
denominator gets its own bank so the ~3.5us HW reciprocal never blocks
the output-projection psum ring.  All DMA uses HWDGE queues (sync for
x/out, scalar for weights/consts) in few large transfers - SWDGE issue
costs ~1us per DMA and serializes on the Pool engine.

HBM traffic per core ~54MB; PE streams ~660us of matmul columns at
~95% occupancy (TimelineSim 708us).  Measured device body time
(3x-unrolled NEFF, chained-dispatch slope): ~607us with the device
un-throttled, ~918us throttled, vs ~1211us (throttled) for the
previous two-pass kernel.  The trn2 terminal alternates between those
power states on a minutes timescale, which dominates run-to-run
variance.
"""

import sys

if "/opt/trn_rl_repo" not in sys.path:
    sys.path.insert(0, "/opt/trn_rl_repo")

import numpy as np

B, S, D, H, KV, HD = 2, 2048, 4096, 32, 8, 128
TPG = 4                 # tensor-parallel groups (x2 data-parallel = 8 cores)
HL = H // TPG           # 8 q heads per core
KVL = KV // TPG         # 2 kv heads per core
FL = HL * HD            # 1024 local features
QT = 512                # q/k/v projection + attention q-tile
NQT = S // QT           # 4
NKT = S // 128          # 16 k-chunks
NDCH = D // 128         # 32 contraction chunks
SCALE = 1.0 / float(np.sqrt(HD))
EXP_BIAS = -2.0         # constant shift inside exp; cancels in softmax
# stream_shuffle mask swapping lanes 0:16 <-> 16:31 within each
# 32-partition quadrant (rope real/imag halves, 16-interleaved layout)
SWAP_MASK = list(range(16, 32)) + list(range(16))

_cache = {}


def _build(causal: bool, reps: int = 1):
    import concourse.mybir as mybir
    import concourse.tile as tile
    from concourse import bacc

    dt = mybir.dt
    f32 = dt.float32
    f16 = dt.float16
    AF = mybir.ActivationFunctionType
    ALU = mybir.AluOpType

    nc = bacc.Bacc()
    xT = nc.dram_tensor("xT", [D, S], f16, kind="ExternalInput")
    wqT = nc.dram_tensor("wqT", [D, FL], f16, kind="ExternalInput")
    wkT = nc.dram_tensor("wkT", [D, KVL * HD], f16, kind="ExternalInput")
    wvT = nc.dram_tensor("wvT", [D, KVL * HD], f16, kind="ExternalInput")
    woP = nc.dram_tensor("woP", [128, 8, HL, QT], f16, kind="ExternalInput")
    onesC = nc.dram_tensor("onesC", [128, 128], f16, kind="ExternalInput")
    cosP = nc.dram_tensor("cosP", [128, S], f16, kind="ExternalInput")
    sinP = nc.dram_tensor("sinP", [128, S], f16, kind="ExternalInput")
    if causal:
        triC = nc.dram_tensor("triC", [128, 128], f16, kind="ExternalInput")
    else:
        maskT = nc.dram_tensor("maskT", [S, S], f16, kind="ExternalInput")
    outp = nc.dram_tensor("outp", [S, D], f16, kind="ExternalOutput")

    with tile.TileContext(nc) as tc:
      for _rep in range(reps):
        with (
            tc.tile_pool(name="const", bufs=1) as constp,
            tc.tile_pool(name="ps", bufs=1, space="PSUM") as psp,
        ):
            ones = constp.tile([128, 128], f16)
            tri = None
            if causal:
                tri = constp.tile([128, 128], f16, name="tri")
            biasT = constp.tile([128, 1], f32)
            nc.vector.memset(biasT, EXP_BIAS)
            # preload the Exp table set during phase A
            warmact = constp.tile([128, 1], f32)
            nc.scalar.activation(warmact, biasT, AF.Exp)
            cosb = constp.tile([128, S], f16)
            sinb = constp.tile([128, S], f16)

            ctx_kv = tc.tile_pool(name="kv", bufs=1)
            kvp = ctx_kv.__enter__()
            kT_sb = [kvp.tile([128, S], f16, name=f"kT{i}")
                     for i in range(KVL)]
            v_sb = [kvp.tile([128, NKT, 128], f16, name=f"v{i}")
                    for i in range(KVL)]
            ctx_q = tc.tile_pool(name="qres", bufs=1)
            qp_ = ctx_q.__enter__()
            q_sb = [qp_.tile([128, S], f16, name=f"q{h}") for h in range(HL)]

            def rope(dst, raw, swp, qt, pool):
                """dst = raw*cos + swp*sin_signed (f16 throughout)."""
                c = cosb[:, qt * QT:(qt + 1) * QT]
                s = sinb[:, qt * QT:(qt + 1) * QT]
                tmp = pool.tile([128, QT], f16, name="ropetmp", tag="ropetmp")
                nc.vector.tensor_tensor(dst, raw, c, ALU.mult)
                nc.vector.tensor_tensor(tmp, swp, s, ALU.mult)
                nc.vector.tensor_tensor(dst, dst, tmp, ALU.add)

            # -------- phase A: fused q/k/v projection + rope --------
            with (
                tc.tile_pool(name="wres", bufs=1) as wp,
                tc.tile_pool(name="x", bufs=1) as xp,
                tc.tile_pool(name="rope", bufs=3) as ropep,
            ):
                wq_res = wp.tile([128, NDCH, FL], f16, name="wqr")
                wk_res = wp.tile([128, NDCH, KVL * HD], f16, name="wkr")
                wv_res = wp.tile([128, NDCH, KVL * HD], f16, name="wvr")
                # weights on the scalar HWDGE queue (x owns the sync
                # queue); few large DMAs amortize ~630ns issue cost.  The
                # first wq group is split so d=0 matmuls start early.
                def load_wq(d0, d1):
                    nc.scalar.dma_start(
                        wq_res[:, d0:d1, :],
                        wqT[d0 * 128:d1 * 128, :].rearrange(
                            "(d p) f -> p d f", p=128))

                # first chunk split by head so the h0/d0 matmul can start
                # as soon as 32KB (not 256KB) has landed
                nc.scalar.dma_start(wq_res[:, 0:1, 0:128],
                                    wqT[0:128, 0:128].rearrange(
                                        "(d p) f -> p d f", p=128))
                nc.scalar.dma_start(wq_res[:, 0:1, 128:FL],
                                    wqT[0:128, 128:FL].rearrange(
                                        "(d p) f -> p d f", p=128))
                load_wq(1, 2)
                load_wq(2, 8)
                for g in range(1, 4):
                    load_wq(g * 8, (g + 1) * 8)
                nc.scalar.dma_start(
                    wk_res, wkT[:, :].rearrange("(d p) f -> p d f", p=128))
                nc.scalar.dma_start(
                    wv_res, wvT[:, :].rearrange("(d p) f -> p d f", p=128))
                nc.scalar.dma_start(ones, onesC[:, :])
                if causal:
                    nc.scalar.dma_start(tri, triC[:, :])
                nc.scalar.dma_start(cosb, cosP[:, :])
                nc.scalar.dma_start(sinb, sinP[:, :])

                for qt in range(NQT):
                    # PSUM tags: A, B are 2-bank tiles; c..f one bank each.
                    # q heads 0..3 live in A/B halves, 4..7 in c..f.
                    qA = psp.tile([128, 2 * QT], f32, name="qA", tag="A")
                    qB = psp.tile([128, 2 * QT], f32, name="qB", tag="B")
                    qsingle = {4 + i: psp.tile([128, QT], f32, name=f"q{4+i}",
                                               tag="cdef"[i])
                               for i in range(4)}

                    def qps(h):
                        if h == 0:
                            return qA[:, 0:QT]
                        if h == 1:
                            return qA[:, QT:2 * QT]
                        if h == 2:
                            return qB[:, 0:QT]
                        if h == 3:
                            return qB[:, QT:2 * QT]
                        return qsingle[h]

                    xts = []
                    for g in range(4):
                        xg = xp.tile([128, 8, QT], f16, tag=f"xg{g}")

                        def load_x(d0, d1):
                            nc.sync.dma_start(
                                xg[:, d0 - g * 8:d1 - g * 8, :],
                                xT[d0 * 128:d1 * 128,
                                   qt * QT:(qt + 1) * QT].rearrange(
                                    "(d p) n -> p d n", p=128))

                        if qt == 0 and g == 0:
                            load_x(0, 1)
                            load_x(1, 2)
                            load_x(2, 8)
                        else:
                            load_x(g * 8, (g + 1) * 8)
                        xts.extend(xg[:, i, :] for i in range(8))
                    for d in range(NDCH):
                        xt = xts[d]
                        for h in range(HL):
                            nc.tensor.matmul(
                                qps(h), wq_res[:, d, h * 128:(h + 1) * 128],
                                xt, start=(d == 0), stop=(d == NDCH - 1))

                    # rope q heads 0..5 first: frees A, B (k/v psum reuse)
                    # and c, d (v2/v3); heads 6,7 roped after the kv pass.
                    def rope_q(h):
                        qraw = ropep.tile([128, QT], f16, tag="qraw")
                        if qt == NQT - 1 and h >= 6:
                            nc.vector.tensor_copy(qraw, qps(h))
                        else:
                            nc.scalar.copy(qraw, qps(h))
                        qswp = ropep.tile([128, QT], f16, tag="qswp")
                        nc.vector.stream_shuffle(qswp, qraw, SWAP_MASK)
                        rope(q_sb[h][:, qt * QT:(qt + 1) * QT],
                             qraw, qswp, qt, ropep)

                    for h in (0, 1, 2, 3, 4, 5):
                        rope_q(h)
                    # k/v projection reusing freed psum banks.  Each
                    # accumulating region owns a full bank (start=True
                    # clears the whole bank).
                    ktile = psp.tile([128, 2 * QT], f32, name="kt", tag="B")
                    kpsum = [ktile[:, i * QT:(i + 1) * QT]
                             for i in range(KVL)]
                    vA = psp.tile([128, 2 * QT], f32, name="vA", tag="A")
                    vc = psp.tile([128, KVL * HD], f32, name="vc", tag="c")
                    vd = psp.tile([128, KVL * HD], f32, name="vd", tag="d")
                    vpsum = [vA[:, 0:KVL * HD], vA[:, QT:QT + KVL * HD],
                             vc, vd]
                    # v before k: the last reader of the low-d x tiles is
                    # then k-proj's early iterations, giving the next qt's
                    # xg0 DMA more lead time
                    for d in range(NDCH):
                        xt = xts[d]
                        for t in range(4):
                            nc.tensor.matmul(
                                vpsum[t], xt[:, t * 128:(t + 1) * 128],
                                wv_res[:, d, :],
                                start=(d == 0), stop=(d == NDCH - 1))
                    for d in range(NDCH):
                        xt = xts[d]
                        for i in range(KVL):
                            nc.tensor.matmul(
                                kpsum[i],
                                wk_res[:, d, i * 128:(i + 1) * 128], xt,
                                start=(d == 0), stop=(d == NDCH - 1))
                    last = qt == NQT - 1
                    for t in range(4):
                        for i in range(KVL):
                            if last:
                                nc.vector.tensor_copy(
                                    v_sb[i][:, qt * 4 + t, :],
                                    vpsum[t][:, i * 128:(i + 1) * 128])
                            else:
                                nc.scalar.copy(
                                    v_sb[i][:, qt * 4 + t, :],
                                    vpsum[t][:, i * 128:(i + 1) * 128])
                    for i in range(KVL):
                        kraw = ropep.tile([128, QT], f16, tag="qraw")
                        if last:
                            nc.vector.tensor_copy(kraw, kpsum[i])
                        else:
                            nc.scalar.copy(kraw, kpsum[i])
                        kswp = ropep.tile([128, QT], f16, tag="qswp")
                        nc.vector.stream_shuffle(kswp, kraw, SWAP_MASK)
                        rope(kT_sb[i][:, qt * QT:(qt + 1) * QT],
                             kraw, kswp, qt, ropep)
                    for h in (6, 7):
                        rope_q(h)

            # -------- phase B: attention + output projection --------
            # PSUM tag plan (phase-A tenants in parens): sp ring = 2-bank
            # tags A (v01), B (k); opsum ring c/d (v2/v3); wpsum/dpsum
            # ring e/f (q-rope h6/h7).
            sp_idx = [0]
            op_idx = [0]
            wp_idx = [0]

            with (
                tc.tile_pool(name="attn", bufs=1) as attnp,
                tc.tile_pool(name="wot", bufs=1) as wotp,
                tc.tile_pool(name="pt", bufs=6) as ptp,
                tc.tile_pool(name="acc", bufs=2) as accp,
                tc.tile_pool(name="rec", bufs=2) as recp,
                tc.tile_pool(name="o4", bufs=4) as op4,
                tc.tile_pool(name="m3", bufs=1 if causal else NKT + 1) as mp3,
            ):
                attnT_sb = [attnp.tile([128, S], f16, name=f"aT{h}")
                            for h in range(HL)]
                wot = [wotp.tile([128, HL, QT], f16, name=f"wo{dd}")
                       for dd in range(8)]
                for dd in range(8):
                    nc.scalar.dma_start(wot[dd], woP[:, dd])

                def p4_chunk(qt, dd):
                    osb = op4.tile([128, 4, QT], f16, tag="osb")
                    for tcn in range(4):
                        sc = qt * 4 + tcn
                        wp_idx[0] ^= 1
                        wpsum = psp.tile([128, QT], f32,
                                         tag="ef"[wp_idx[0]])
                        for f in range(HL):
                            nc.tensor.matmul(
                                wpsum,
                                attnT_sb[f][:, sc * 128:(sc + 1) * 128],
                                wot[dd][:, f, :],
                                start=(f == 0), stop=(f == HL - 1))
                        if tcn % 2 == 0:
                            nc.vector.tensor_copy(osb[:, tcn, :], wpsum)
                        else:
                            nc.scalar.copy(osb[:, tcn, :], wpsum)
                    nc.sync.dma_start(
                        outp[qt * QT:(qt + 1) * QT,
                             dd * QT:(dd + 1) * QT].rearrange(
                            "(tcn p) n -> p tcn n", p=128), osb)

                def p3_head(qt, h, mtiles):
                    kvh = h // (HL // KVL)
                    nkt = 4 * (qt + 1) if causal else NKT
                    if qt == 0 and causal:
                        op_idx[0] = (op_idx[0] + 1) % 3
                        opsum = psp.tile([128, QT], f32,
                                         tag="cde"[op_idx[0]])
                    else:
                        opsum = psp.tile([128, QT], f32, tag="c")
                    ptacc = accp.tile([128, QT], f16, tag="pa")
                    for g in range(nkt // 2):
                        sp_idx[0] ^= 1
                        sp = psp.tile([128, 2 * QT], f32, tag="AB"[sp_idx[0]])
                        pt = ptp.tile([128, 2 * QT], f16, tag="pt")
                        zs = [max((2 * g + c - qt * 4) if causal else -1, 0)
                              * 128 for c in range(2)]
                        # Place each chunk's valid region [z:512] so the two
                        # regions abut (each within one psum bank) -> one
                        # exp per group even for the diagonal groups.
                        if zs == [0, 128]:
                            offs, lo, hi = (QT, 128), 128, 2 * QT
                        elif zs == [256, 384]:
                            offs, lo, hi = (QT, 384), 384, QT + 256
                        else:       # [0, 0]
                            offs, lo, hi = (0, QT), 0, 2 * QT
                        for c in range(2):
                            kt = 2 * g + c
                            z = zs[c]
                            nc.tensor.matmul(
                                sp[:, offs[c]:offs[c] + QT - z],
                                kT_sb[kvh][:, kt * 128:(kt + 1) * 128],
                                q_sb[h][:, qt * QT + z:(qt + 1) * QT],
                                start=True, stop=True)
                            if not causal:
                                nc.vector.tensor_tensor(
                                    sp[:, offs[c]:offs[c] + QT],
                                    sp[:, offs[c]:offs[c] + QT],
                                    mtiles[kt], ALU.add)
                        nc.scalar.activation(
                            pt[:, lo:hi], sp[:, lo:hi], AF.Exp,
                            bias=biasT, scale=SCALE)
                        for c in range(2):
                            kt = 2 * g + c
                            p = kt - qt * 4 if causal else -1
                            z = zs[c]
                            if causal and p >= 0:
                                nc.vector.tensor_tensor(
                                    pt[:, offs[c]:offs[c] + 128],
                                    pt[:, offs[c]:offs[c] + 128],
                                    tri, ALU.mult)
                            nc.tensor.matmul(
                                opsum[:, z:], v_sb[kvh][:, kt, :],
                                pt[:, offs[c]:offs[c] + QT - z],
                                start=(kt == 0), stop=(kt == nkt - 1))
                            # ptacc on Pool (idle engine, SBUF-only) except
                            # qt==0 where its latency chain is exposed
                            acceng = nc.vector if qt == 0 else nc.gpsimd
                            if kt == 0:
                                acceng.tensor_copy(
                                    ptacc, pt[:, offs[c]:offs[c] + QT])
                            else:
                                acceng.tensor_tensor(
                                    ptacc[:, z:], ptacc[:, z:],
                                    pt[:, offs[c]:offs[c] + QT - z],
                                    ALU.add)
                    # dpsum must not share a psum ring with P4's wpsum:
                    # the ~3.5us HW reciprocal reading it would stall P4.
                    # qt>0: opsum=c, dpsum=d; qt==0 (no P4 running):
                    # opsum alternates c/d, dpsum alternates e/f.
                    if qt == 0 and causal:
                        dpsum = psp.tile([128, QT], f32, tag="f",
                                         name="dp0")
                    else:
                        dpsum = psp.tile([128, QT], f32, tag="d")
                    nc.tensor.matmul(dpsum, ones, ptacc,
                                     start=True, stop=True)
                    rec = recp.tile([128, QT], f32, tag="rec")
                    # ~5x faster than the HW reciprocal op; denominators are
                    # well inside its safe range (positive, >= exp(bias))
                    nc.vector.reciprocal_approx_fast(rec, dpsum)
                    nc.vector.tensor_tensor(
                        attnT_sb[h][:, qt * QT:(qt + 1) * QT],
                        opsum, rec, ALU.mult)

                for qt in range(NQT):
                    mtiles = []
                    if not causal:
                        for kt in range(NKT):
                            mt = mp3.tile([128, QT], f16, tag="mt")
                            nc.sync.dma_start(
                                mt, maskT[kt * 128:(kt + 1) * 128,
                                          qt * QT:(qt + 1) * QT])
                            mtiles.append(mt)
                    for h in range(HL):
                        p3_head(qt, h, mtiles)
                        if qt >= 1:
                            p4_chunk(qt - 1, h)
                for dd in range(8):
                    p4_chunk(NQT - 1, dd)

            ctx_q.__exit__(None, None, None)
            ctx_kv.__exit__(None, None, None)
    nc.finalize()
    return nc


# partition p = 32q + l: l<16 -> real part of pair 16q+l (elem 2j);
# l>=16 -> imag part of pair 16q+(l-16) (elem 2j+1)
_PERM = np.array([2 * (16 * (p // 32) + (p % 32) % 16) + (1 if p % 32 >= 16 else 0)
                  for p in range(HD)])
# row p of cosP/sinP uses rope pair index j(p)
_JROW = np.array([16 * (p // 32) + (p % 32) % 16 for p in range(HD)])
_SSGN = np.array([-1.0 if p % 32 < 16 else 1.0 for p in range(HD)],
                 dtype=np.float32)


def _is_causal(mask):
    if mask.shape != (S, S):
        return False
    tril = np.tril(np.ones((S, S), dtype=bool))
    if not np.all(mask[tril] == 0.0):
        return False
    return bool(np.all(mask[~tril] <= -1e8))


def kernel(x, wq, wk, wv, wo, cos, sin, mask, start_pos):
    from concourse import bass_utils

    x = np.asarray(x, np.float32)
    wq = np.asarray(wq, np.float32)
    wk = np.asarray(wk, np.float32)
    wv = np.asarray(wv, np.float32)
    wo = np.asarray(wo, np.float32)
    cos = np.asarray(cos, np.float32)
    sin = np.asarray(sin, np.float32)
    mask = np.asarray(mask, np.float32)

    causal = _is_causal(mask)
    key = causal
    if key not in _cache:
        _cache[key] = _build(causal)
    nc = _cache[key]

    ones_c = np.ones((128, 128), np.float16)
    cosP = np.ascontiguousarray(cos.T[_JROW]).astype(np.float16)
    sinP = np.ascontiguousarray(
        sin.T[_JROW] * _SSGN[:, None]).astype(np.float16)
    if causal:
        k_idx = np.arange(128)[:, None]
        q_idx = np.arange(128)[None, :]
        triC = (k_idx <= q_idx).astype(np.float16)
    else:
        maskT = np.clip(mask.T * np.float64(np.sqrt(HD)),
                        -60000.0, 60000.0).astype(np.float16)
        maskT = np.ascontiguousarray(maskT)

    in_maps = []
    shard_data = []
    for j in range(TPG):
        wq_j = wq[j * FL:(j + 1) * FL].reshape(HL, HD, D)[:, _PERM, :]
        wqTj = np.ascontiguousarray(wq_j.reshape(FL, D).T, np.float16)
        wk_j = wk[j * KVL * HD:(j + 1) * KVL * HD].reshape(
            KVL, HD, D)[:, _PERM, :]
        wkTj = np.ascontiguousarray(wk_j.reshape(KVL * HD, D).T, np.float16)
        wvTj = np.ascontiguousarray(
            wv[j * KVL * HD:(j + 1) * KVL * HD].T, np.float16)
        # woP[p, dd, fo, n] = wo[dd*512+n, j*1024 + fo*128 + p]
        wo_j = wo[:, j * FL:(j + 1) * FL]          # [4096, 1024]
        woPj = np.ascontiguousarray(
            wo_j.reshape(8, QT, HL, 128).transpose(3, 0, 2, 1)
        ).astype(np.float16)
        shard_data.append((wqTj, wkTj, wvTj, woPj))

    xTs = [np.ascontiguousarray(x[b].T, np.float16) for b in range(B)]
    for c in range(8):
        b, j = divmod(c, TPG)
        wqTj, wkTj, wvTj, woPj = shard_data[j]
        m = {
            "xT": xTs[b], "wqT": wqTj, "wkT": wkTj, "wvT": wvTj,
            "woP": woPj, "cosP": cosP, "sinP": sinP, "onesC": ones_c,
        }
        if causal:
            m["triC"] = triC
        else:
            m["maskT"] = maskT
        in_maps.append(m)

    global _last_in_maps
    _last_in_maps = in_maps
    res = bass_utils.run_bass_kernel_spmd(nc, in_maps, core_ids=list(range(8)))
    out = np.zeros((B, S, D), np.float32)
    for c in range(8):
        b = c // TPG
        out[b] += res.results[c]["outp"].astype(np.float32)
    return out

